# revision 8
# baseline (speedup 1.0000x reference)
"""kNN-retrieval kernel for Trainium2 (8 NeuronCores).

Pipeline:
  host:   transpose obs into a feature-major layout, cast to bf16
  device: per core, stream 1/8th of obs and compute, per row, the dot
          product with the normalized query (xn) and the squared norm --
          both as TensorEngine matmuls against tiny block-diagonal
          weights (features live on the partition axis, so the PE's
          partition contraction does the 32-wide reductions).
  host:   approx cosine sim for all 2M rows, take top-8192 candidates,
          recompute their cosine distance exactly in fp32, take the true
          top-k, run the tiny MLP + softmax + weighted action sum.

The bf16 approximation is safe for selection: worst-case sim error is
~1e-2 while the rank-128 vs rank-8192 sim gap on 2M gaussian rows is
~0.09, so the exact top-128 is always inside the top-8192 candidates.
"""

import sys

sys.path.insert(0, "/opt/trn_rl_repo")

import numpy as np
import ml_dtypes

import bass_rust
import concourse.bass as bass
import concourse.mybir as mybir
from concourse.bass_utils import run_bass_kernel_spmd
from concourse.tile import TileContext

BF16 = ml_dtypes.bfloat16

N = 2_000_000
D = 32
N_CORES = 8
ROWS_PER_CORE = N // N_CORES          # 250_000
CHUNKS = 4                            # feature-major row chunks per core
S = ROWS_PER_CORE // CHUNKS           # 62_500 columns per chunk
TILE_COLS = 2_500                     # SBUF tile width
MM_COLS = 500                         # matmul free dim (<=512 fp32 PSUM bank)
DVE_COLS = 1_500                      # square-pass columns done on VectorE
EPS = np.float32(1e-8)
TOPK = 128
CAND = 8_192                          # host re-rank depth

MAX_WAITS = 1  # walrus in this container allows 1 sync wait per instruction


def _split_wide_waits(nc):
    """Hoist excess per-instruction sem waits onto NoOps inserted just
    before, on the same engine (each engine executes its bb-subsequence in
    order, so this is semantically identical)."""
    uid = 0
    for f in nc.m.functions:
        for bb in f.blocks:
            out = []
            changed = False
            for ins in bb.instructions:
                si = ins.sync_info
                waits = list(si.on_wait) if si and si.on_wait else []
                if len(waits) > MAX_WAITS:
                    changed = True
                    extra, keep = waits[:-MAX_WAITS], waits[-MAX_WAITS:]
                    for i in range(0, len(extra), MAX_WAITS):
                        nop = mybir.InstNoOp(
                            name=f"waitsplit_nop_{uid}", ins=[], outs=[]
                        )
                        uid += 1
                        nop.engine = ins.engine
                        nop.sync_info = bass_rust.SyncInfo(
                            on_wait=extra[i : i + MAX_WAITS], on_update=[]
                        )
                        out.append(nop)
                    ins.sync_info = bass_rust.SyncInfo(
                        on_wait=keep, on_update=list(si.on_update or [])
                    )
                out.append(ins)
            if changed:
                bb.instructions = out


def build_program():
    """Per-core SPMD program: obsT [128, S] bf16 (partition = 32*chunk +
    feature), wts [128, 8] bf16 (cols 0-3 xn block-diag, cols 4-7 ones
    block-diag) -> res [8, S] f32 (rows 0-3 dots, rows 4-7 sq-norms)."""
    nc = bass.Bass("TRN2", target_bir_lowering=False, debug=False,
                   num_devices=N_CORES)
    obsT = nc.dram_tensor("obsT", [128, S], mybir.dt.bfloat16,
                          kind="ExternalInput").ap()
    wts = nc.dram_tensor("wts", [128, 8], mybir.dt.bfloat16,
                         kind="ExternalInput").ap()
    res = nc.dram_tensor("res", [8, S], mybir.dt.bfloat16,
                         kind="ExternalOutput").ap()

    n_chunks = S // MM_COLS          # 125 matmul column-chunks per core
    cpt = TILE_COLS // MM_COLS       # 5 chunks per obs tile
    n_super = (n_chunks + 3) // 4    # supertiles of up to 4 chunks
    with TileContext(nc) as tc:
        with (
            tc.tile_pool(name="wp", bufs=1) as wp,
            tc.tile_pool(name="op", bufs=4) as op,
            tc.tile_pool(name="qp", bufs=4) as qp,
            tc.tile_pool(name="sp", bufs=4) as sp,
            tc.tile_pool(name="pp", bufs=3, space="PSUM") as pp,
        ):
            w = wp.tile([128, 8], mybir.dt.bfloat16)
            nc.sync.dma_start(out=w, in_=wts)

            ots, sts = {}, {}

            def ensure_tile(t):
                if t in ots:
                    return
                ot = op.tile([128, TILE_COLS], mybir.dt.bfloat16, tag="ot")
                nc.sync.dma_start(
                    out=ot, in_=obsT[:, t * TILE_COLS : (t + 1) * TILE_COLS]
                )
                st = qp.tile([128, TILE_COLS], mybir.dt.bfloat16, tag="st")
                nc.vector.tensor_mul(
                    out=st[:, :DVE_COLS], in0=ot[:, :DVE_COLS],
                    in1=ot[:, :DVE_COLS],
                )
                nc.scalar.square(out=st[:, DVE_COLS:], in_=ot[:, DVE_COLS:])
                ots[t], sts[t] = ot, st

            for s in range(n_super):
                chunks = range(4 * s, min(4 * s + 4, n_chunks))
                for t in sorted({c // cpt for c in chunks}):
                    ensure_tile(t)
                # pack up to 4 matmul outputs per PSUM bank at base
                # partitions 0/32/64/96; one [100, 500] copy drains all 4
                ps_d = pp.tile([100, MM_COLS], mybir.dt.float32, tag="psd")
                ps_n = pp.tile([100, MM_COLS], mybir.dt.float32, tag="psn")
                for i, c in enumerate(chunks):
                    t, off = c // cpt, (c % cpt) * MM_COLS
                    b = 32 * i
                    nc.tensor.matmul(
                        ps_d[b : b + 4, :], w[:, 0:4],
                        ots[t][:, off : off + MM_COLS],
                        start=True, stop=True, tile_position=(0, b),
                    )
                    nc.tensor.matmul(
                        ps_n[b : b + 4, :], w[:, 4:8],
                        sts[t][:, off : off + MM_COLS],
                        start=True, stop=True, tile_position=(0, b),
                    )
                np_used = 32 * (len(chunks) - 1) + 4
                sd = sp.tile([100, MM_COLS], mybir.dt.bfloat16, tag="sd")
                sn = sp.tile([100, MM_COLS], mybir.dt.bfloat16, tag="sn")
                nc.vector.tensor_copy(sd[:np_used, :], ps_d[:np_used, :])
                nc.scalar.copy(sn[:np_used, :], ps_n[:np_used, :])
                for i, c in enumerate(chunks):
                    b, col = 32 * i, c * MM_COLS
                    nc.sync.dma_start(
                        out=res[0:4, col : col + MM_COLS],
                        in_=sd[b : b + 4, :],
                    )
                    nc.sync.dma_start(
                        out=res[4:8, col : col + MM_COLS],
                        in_=sn[b : b + 4, :],
                    )

    _split_wide_waits(nc)
    return nc


_nc_cache = None


def _get_program():
    global _nc_cache
    if _nc_cache is None:
        _nc_cache = build_program()
    return _nc_cache


def _query_normalize(x32):
    s = np.float32(np.sum(x32 * x32, dtype=np.float32))
    return (x32 * np.float32(1.0 / np.sqrt(s + EPS))).astype(np.float32)


def prep_inputs(obs, x):
    """Host-side shard prep: per-core feature-major bf16 obs + weights."""
    obs32 = np.ascontiguousarray(np.asarray(obs, dtype=np.float32))
    x32 = np.asarray(x, dtype=np.float32)
    xn = _query_normalize(x32)

    obs_bf = obs32.astype(BF16)
    # [core, chunk, row-in-chunk, feat] -> [core, chunk, feat, row-in-chunk]
    obsT = np.ascontiguousarray(
        obs_bf.reshape(N_CORES, CHUNKS, S, D).transpose(0, 1, 3, 2)
    ).reshape(N_CORES, 128, S)

    wts = np.zeros((128, 8), np.float32)
    for q in range(CHUNKS):
        wts[32 * q : 32 * (q + 1), q] = xn
        wts[32 * q : 32 * (q + 1), 4 + q] = 1.0
    wts_bf = wts.astype(BF16)

    in_maps = [{"obsT": obsT[c], "wts": wts_bf} for c in range(N_CORES)]
    return in_maps, obs32, xn


def postprocess(results, obs32, xn, acs, w_in, b_in, w2, b2, w3, b3,
                w_out, b_out, k):
    """Host: candidate selection, exact re-rank, MLP, weighted sum."""
    k = int(k)
    dots = np.concatenate(
        [r["res"][0:4].reshape(-1).astype(np.float32) for r in results]
    )
    nrms = np.concatenate(
        [r["res"][4:8].reshape(-1).astype(np.float32) for r in results]
    )
    approx_sim = dots / np.sqrt(nrms + EPS)

    n_cand = max(CAND, 4 * k)
    cand = np.argpartition(-approx_sim, n_cand)[:n_cand]

    sub = obs32[cand]
    sub_nrm = np.float32(1.0) / np.sqrt(
        np.sum(sub * sub, axis=1, dtype=np.float32) + EPS
    )
    cos_sim = (sub @ xn) * sub_nrm
    cos_dist = np.float32(1.0) - cos_sim

    order = np.lexsort((cand, cos_dist))[:k]
    k_dist = cos_dist[order].astype(np.float32)
    idx = cand[order]

    acs32 = np.asarray(acs, dtype=np.float32)
    k_actions = acs32[idx]

    w_in = np.asarray(w_in, dtype=np.float32)
    b_in = np.asarray(b_in, dtype=np.float32)
    w2 = np.asarray(w2, dtype=np.float32)
    b2 = np.asarray(b2, dtype=np.float32)
    w3 = np.asarray(w3, dtype=np.float32)
    b3 = np.asarray(b3, dtype=np.float32)
    w_out = np.asarray(w_out, dtype=np.float32)
    b_out = np.asarray(b_out, dtype=np.float32)

    h = np.maximum(k_dist @ w_in.T + b_in, np.float32(0.0))
    h = np.maximum(h @ w2.T + b2, np.float32(0.0))
    h = np.maximum(h @ w3.T + b3, np.float32(0.0))
    logits = h @ w_out.T + b_out
    z = np.exp(logits - logits.max())
    weights = (z / z.sum()).astype(np.float32)

    out = np.sum(weights[:, None] * k_actions, axis=0, keepdims=True)
    return out.astype(np.float32)


def kernel(obs, acs, x, w_in, b_in, w2, b2, w3, b3, w_out, b_out, k):
    nc = _get_program()
    in_maps, obs32, xn = prep_inputs(obs, x)
    results = run_bass_kernel_spmd(
        nc, in_maps, core_ids=list(range(N_CORES))
    ).results
    return postprocess(results, obs32, xn, acs, w_in, b_in, w2, b2, w3, b3,
                       w_out, b_out, k)
